# revision 3
# baseline (speedup 1.0000x reference)
"""Trainium2 Bass kernel for nn_AttentionSE3 (graph attention message passing).

Strategy (edge/graph parallel, fully host-prepped ELL layout):
- Attention is a segment softmax over incoming edges of each dst node.  Logits are
  dot(k_edge, q_dst)/sqrt(128) with k,q ~ N(0,1): |logit| <~ 2, so the max-subtraction
  is dropped (softmax is shift-invariant; exp() never overflows here) and
  out[n] = sum_e exp(logit_e) * v_e / sum_e exp(logit_e).
- Host sorts nodes by in-degree, packs them into 128-node blocks, and pads each
  block's per-node edge lists to the block max degree D (degree sorting makes the
  padding small).  Blocks are dealt round-robin to the 8 cores; the per-group
  capacity is the max over the 8 cores so EVERY core runs the same static program
  (no collectives: no node's edges ever span two cores).
- Per (node, d) "slot" the host gathers the edge's key row [128] and value row [96]
  (zero for padding).  A padded slot contributes exactly exp(0)=1 to the softmax
  denominator, so the device subtracts a per-node pad count (exact correction).
  Zero-degree nodes get pad_count = D-1 so the denominator is exactly 1 and the
  output row is 0, matching segment_sum semantics.
- ALL compute stays on VectorE + ScalarE.  GPSIMD shares an SBUF port with
  VectorE; measured on HW, a DVE tensor_tensor slows down 7-9x while any GPSIMD
  tensor op runs, so offloading elementwise work to GPSIMD is a large net loss.
- Key columns are stored kk-major ([16 dk, 8 heads] per slot) so the dk-reduction
  of k*q is a 4-level pairwise-halving tree with long contiguous runs: every level
  runs in bf16 2x mode (tensor_reduce is stuck at 1x and strided reduces are
  ~1.8 cyc/elem; the tree is ~2.5x faster).  The tree runs in place inside the
  k-tile (the multiply k *= q is also in place).
- Value columns are [cx(12), h(8)] per slot so the expw broadcast in the
  weighting multiply lands on a middle AP dim (2x mode, in place on the v-tile).
  The d-reduction halves in place (flat contiguous adds, 2x) while the running
  depth is even and finishes with one strided reduce over the odd remainder.
  The host rounds each group capacity D up a little when that makes the halving
  chain cheaper (cost model below) - typically to a multiple of 4 or 8.
- Normalization is one deferred wide pass at the end; output accumulates in SBUF
  and is stored with one DMA.
"""

import numpy as np

import concourse.bacc as bacc
import concourse.mybir as mybir
from concourse import tile
from concourse.bass_utils import run_bass_kernel_spmd

try:
    import ml_dtypes
    BF16_NP = np.dtype(ml_dtypes.bfloat16)
except ImportError:  # pragma: no cover
    BF16_NP = None

N_NODES = 50000
H = 8
P = 128  # nodes per block
N_CORES = 8
SCALE = float(1.0 / np.sqrt(128.0))
F32 = mybir.dt.float32

# key columns permuted from [h(8), kk(16)] to [kk(16), h(8)]: the dk-halving tree
# then operates on contiguous 64/32/16/8-elem runs per d-block.
PERM_K = np.arange(128).reshape(8, 16).T.reshape(-1)  # new_col kk*8+h -> old h*16+kk
# value columns permuted from [h(8), cx(12)] to [cx(12), h(8)].
PERM_V = np.arange(96).reshape(8, 12).T.reshape(-1)  # new_col cx*8+h -> old h*12+cx
PERM_V_INV = np.argsort(PERM_V)

# measured per-element DVE cycle costs (bf16, clean APs, incl. typical overhead)
_C_HALVE = 0.7    # pairwise add, flat contiguous
_C_SRED = 1.82    # strided tensor_reduce
_C_SLOT = 270.0   # total per-partition cycles added by one extra pad slot


def _chain_cost(D):
    """Per-partition cycle cost of the d-reduction for capacity D."""
    c, x = 0.0, D
    while x > 1 and x % 2 == 0:
        x //= 2
        c += _C_HALVE * 96 * x
    if x > 1:
        c += _C_SRED * 96 * x
    return c


def _round_D(D):
    """Pick capacity >= D minimizing padding + d-reduction cost."""
    best, best_c = D, _chain_cost(D)
    for Dp in range(D + 1, D + 8):
        c = _C_SLOT * (Dp - D) + _chain_cost(Dp)
        if c < best_c:
            best, best_c = Dp, c
    return best


# ---------------------------------------------------------------- host prep

def prepare(value, key, query0, query1, edge_index, n_nodes=N_NODES, n_cores=N_CORES):
    """Build per-core padded ELL shards.  Returns (in_maps, meta)."""
    value = np.asarray(value, dtype=np.float32)
    key = np.asarray(key, dtype=np.float32)
    query0 = np.asarray(query0, dtype=np.float32)
    query1 = np.asarray(query1, dtype=np.float32)
    n_edges = key.shape[0]

    dst = np.asarray(edge_index[1], dtype=np.int64)
    deg = np.bincount(dst, minlength=n_nodes).astype(np.int64)
    n_pad = -(-n_nodes // (P * n_cores)) * (P * n_cores)  # round up to 1024
    deg_pad = np.concatenate([deg, np.zeros(n_pad - n_nodes, dtype=np.int64)])
    nb = n_pad // P
    ng = nb // n_cores

    order = np.argsort(deg_pad, kind="stable")  # node ids, degree-ascending
    degs_o = deg_pad[order]

    blk_max = degs_o.reshape(nb, P).max(axis=1)
    D_eff = np.maximum(blk_max.reshape(ng, n_cores).max(axis=1), 1).astype(np.int64)
    D_eff = np.array([_round_D(int(d)) for d in D_eff], dtype=np.int64)
    off = np.concatenate([[0], np.cumsum(P * D_eff)]).astype(np.int64)
    S = int(off[-1])  # slots per core

    pos = np.arange(n_pad)
    block = pos // P
    g_of = block // n_cores
    core_of = block % n_cores
    row = pos % P
    Dg = D_eff[g_of]
    base = off[g_of] + row * Dg

    edge_order = np.argsort(dst, kind="stable")
    starts = np.concatenate([[0], np.cumsum(deg)])

    pp = np.repeat(pos, degs_o)
    cum0 = np.concatenate([[0], np.cumsum(degs_o)])[:-1]
    d_idx = np.arange(n_edges) - np.repeat(cum0, degs_o)
    node_of_pp = order[pp]
    edge_ids = edge_order[starts[node_of_pp] + d_idx]
    slot_global = core_of[pp] * S + base[pp] + d_idx

    kp = np.zeros((n_cores * S, 128), dtype=np.float32)
    kp[slot_global] = key[:, PERM_K][edge_ids]
    vp = np.zeros((n_cores * S, 96), dtype=np.float32)
    vp[slot_global] = value.reshape(n_edges, 96)[:, PERM_V][edge_ids]
    kp = kp.reshape(n_cores, S, 128)
    vp = vp.reshape(n_cores, S, 96)

    qfull = np.concatenate([query0, query1], axis=-1).reshape(n_nodes, 128)
    q_pad = np.zeros((n_pad, 128), dtype=np.float32)
    q_pad[:n_nodes] = qfull[:, PERM_K] if True else qfull
    q_sorted = q_pad[order].reshape(nb, P, 128)

    pc = (Dg - degs_o).astype(np.float32)
    zero_deg = degs_o == 0
    pc[zero_deg] = (Dg[zero_deg] - 1).astype(np.float32)
    pc_sorted = pc.reshape(nb, P)

    dt = BF16_NP
    kp = kp.astype(dt)
    vp = vp.astype(dt)
    in_maps = []
    for c in range(n_cores):
        # pre-tiled layouts: q [128, ng*128], pc [128, ng*H]
        q_c = np.ascontiguousarray(
            q_sorted[c::n_cores].transpose(1, 0, 2).reshape(P, ng * 128)).astype(dt)
        pc_c = np.repeat(np.ascontiguousarray(pc_sorted[c::n_cores].T), H, axis=1)
        in_maps.append({"kp": kp[c], "vp": vp[c], "q": q_c, "pc": pc_c})

    meta = dict(D_eff=D_eff, off=off, S=S, NG=ng, NB=nb, order=order,
                n_nodes=n_nodes, n_pad=n_pad)
    return in_maps, meta


def unshard_output(out_cores, meta):
    """out_cores: list of [128, NG*96] -> [n_nodes, 32, 3]."""
    ng, nb = meta["NG"], meta["NB"]
    n_cores = len(out_cores)
    order, n_nodes, n_pad = meta["order"], meta["n_nodes"], meta["n_pad"]
    out_sorted = np.zeros((nb, P, 96), dtype=np.float32)
    for c in range(n_cores):
        out_sorted[c::n_cores] = (
            out_cores[c].reshape(P, ng, 96).transpose(1, 0, 2))
    out_sorted = out_sorted.reshape(n_pad, 96)[:, PERM_V_INV]
    out_full = np.zeros((n_nodes, 96), dtype=np.float32)
    mask = order < n_nodes
    out_full[order[mask]] = out_sorted[mask]
    return out_full.reshape(n_nodes, 32, 3)


# ---------------------------------------------------------------- bass kernel

def build(D_eff, S, NG, n_cores=N_CORES):
    D_eff = [int(d) for d in D_eff]
    off = np.concatenate([[0], np.cumsum([P * d for d in D_eff])]).astype(np.int64)

    nc = bacc.Bacc("TRN2", target_bir_lowering=False, debug=False,
                   num_devices=n_cores)
    DT = mybir.dt.bfloat16
    kp = nc.declare_dram_parameter("kp", [S, 128], DT, isOutput=False)
    vp = nc.declare_dram_parameter("vp", [S, 96], DT, isOutput=False)
    q = nc.declare_dram_parameter("q", [P, NG * 128], DT, isOutput=False)
    pc = nc.declare_dram_parameter("pc", [P, NG * H], F32, isOutput=False)
    out = nc.declare_dram_parameter("out", [P, NG * 96], F32, isOutput=True)

    mult = mybir.AluOpType.mult
    add = mybir.AluOpType.add
    AX = mybir.AxisListType.X

    with tile.TileContext(nc) as tc:
        with tc.tile_pool(name="res", bufs=1) as res, \
             tc.tile_pool(name="work", bufs=3) as work, \
             tc.tile_pool(name="small", bufs=4) as small:
            q_sb = res.tile([P, NG * 128], DT)
            nc.sync.dma_start(q_sb[:], q[:])
            pc_sb = res.tile([P, NG * H], F32)
            nc.sync.dma_start(pc_sb[:], pc[:])
            out_sb = res.tile([P, NG * 96], F32)
            ss_all = res.tile([P, NG * H], F32)

            for g in range(NG):
                D = D_eff[g]
                s0 = int(off[g])
                kt = work.tile([P, D * 128], DT, tag="kt")
                nc.sync.dma_start(
                    kt[:], kp[s0:s0 + P * D, :].rearrange("(n d) f -> n (d f)", n=P))
                vt = work.tile([P, D * 96], DT, tag="vt")
                nc.sync.dma_start(
                    vt[:], vp[s0:s0 + P * D, :].rearrange("(n d) f -> n (d f)", n=P))

                # k *= q in place (q broadcast over d: outer stride-0, bf16 2x)
                qb = (q_sb[:, g * 128:(g + 1) * 128]
                      .unsqueeze(1).broadcast_to([P, D, 128]))
                k3 = kt[:].rearrange("n (d f) -> n d f", d=D)
                nc.vector.tensor_tensor(out=k3, in0=k3, in1=qb, op=mult)

                # dk-halving tree in place: columns are (kk, h) so each level adds
                # two contiguous runs within every d-block (64,32,16-elem runs),
                # the last level writes the compact logits tile.
                for X in (64, 32, 16):
                    kv = kt[:].rearrange("n (d f) -> n d f", d=D)
                    nc.vector.tensor_tensor(
                        out=kv[:, :, :X], in0=kv[:, :, :X], in1=kv[:, :, X:2 * X],
                        op=add)
                lg = small.tile([P, D * H], DT, tag="lg")
                kv = kt[:].rearrange("n (d f) -> n d f", d=D)
                nc.vector.tensor_tensor(
                    out=lg[:].rearrange("n (d h) -> n d h", d=D),
                    in0=kv[:, :, :8], in1=kv[:, :, 8:16], op=add)

                # expw = exp(scale * logits)  (ScalarE; contiguous)
                ew = small.tile([P, D * H], DT, tag="ew")
                nc.scalar.activation(out=ew[:], in_=lg[:],
                                     func=mybir.ActivationFunctionType.Exp,
                                     scale=SCALE)

                # v *= expw in place; v columns are [cx, h] so the expw broadcast
                # is on the middle dim and the inner stays contiguous (2x mode)
                eb = (ew[:].rearrange("n (d h) -> n d h", d=D)
                      .unsqueeze(2).broadcast_to([P, D, 12, H]))
                v4 = vt[:].rearrange("n (d c h) -> n d c h", d=D, c=12)
                nc.vector.tensor_tensor(out=v4, in0=v4, in1=eb, op=mult)

                # segment sum of expw over d: one in-place halve (wv already
                # consumed ew) if D is even, then a strided reduce
                r = D
                if r > 1 and r % 2 == 0:
                    r //= 2
                    nc.vector.tensor_tensor(
                        out=ew[:, :r * H], in0=ew[:, :r * H], in1=ew[:, r * H:],
                        op=add)
                nc.vector.tensor_reduce(
                    out=ss_all[:, g * H:(g + 1) * H],
                    in_=ew[:, :r * H].rearrange("n (d h) -> n h d", d=r),
                    axis=AX, op=add)

                # d-reduction of weighted values: in-place flat halving while
                # even, strided reduce over the odd remainder -> out_sb
                og = out_sb[:, g * 96:(g + 1) * 96]
                r = D
                while r > 1 and r % 2 == 0:
                    r //= 2
                    if r == 1:
                        nc.vector.tensor_tensor(
                            out=og, in0=vt[:, :96], in1=vt[:, 96:192], op=add)
                    else:
                        nc.vector.tensor_tensor(
                            out=vt[:, :r * 96], in0=vt[:, :r * 96],
                            in1=vt[:, r * 96:2 * r * 96], op=add)
                if r > 1:
                    nc.vector.tensor_reduce(
                        out=og, in_=vt[:, :r * 96].rearrange("n (d c) -> n c d", d=r),
                        axis=AX, op=add)

            # one wide deferred normalization pass
            dn_all = res.tile([P, NG * H], F32)
            nc.vector.tensor_sub(out=dn_all[:], in0=ss_all[:], in1=pc_sb[:])
            rs_all = res.tile([P, NG * H], F32)
            nc.vector.reciprocal(out=rs_all[:], in_=dn_all[:])
            out2 = res.tile([P, NG * 96], F32)
            nc.vector.tensor_tensor(
                out=out2[:].rearrange("n (g c h) -> n g c h", g=NG, c=12),
                in0=out_sb[:].rearrange("n (g c h) -> n g c h", g=NG, c=12),
                in1=(rs_all[:].rearrange("n (g h) -> n g h", g=NG)
                     .unsqueeze(2).broadcast_to([P, NG, 12, H])),
                op=mult)

            nc.sync.dma_start(out[:], out2[:])

    nc.compile()
    return nc


# ---------------------------------------------------------------- entry point

LAST_RESULT = None  # BassKernelResults of the most recent run (for test harness)


def kernel(value, key, query0, query1, edge_index):
    global LAST_RESULT
    import os
    in_maps, meta = prepare(value, key, query0, query1, edge_index)
    nc = build(meta["D_eff"], meta["S"], meta["NG"])
    res = run_bass_kernel_spmd(nc, in_maps, list(range(N_CORES)),
                               tmpdir=os.environ.get("BASS_SPMD_TMPDIR"))
    LAST_RESULT = res
    out_cores = [res.results[c]["out"] for c in range(N_CORES)]
    return unshard_output(out_cores, meta)


# revision 4
# speedup vs baseline: 1.0957x; 1.0957x over previous
"""Trainium2 Bass kernel for nn_AttentionSE3 (graph attention message passing).

Strategy (edge/graph parallel, fully host-prepped ELL layout):
- Attention is a segment softmax over incoming edges of each dst node.  Logits are
  dot(k_edge, q_dst)/sqrt(128) with k,q ~ N(0,1): |logit| <~ 2, so the max-subtraction
  is dropped (softmax is shift-invariant; exp() never overflows here) and
  out[n] = sum_e exp(logit_e) * v_e / sum_e exp(logit_e).
- Host sorts nodes by in-degree, packs them into 128-node blocks, and pads each
  block's per-node edge lists to the block max degree D (degree sorting makes the
  padding small).  Blocks are dealt round-robin to the 8 cores; the per-group
  capacity is the max over the 8 cores so EVERY core runs the same static program
  (no collectives: no node's edges ever span two cores).
- Per (node, d) "slot" the host gathers the edge's key row [128] and value row [96]
  (zero for padding).  A padded slot contributes exactly exp(0)=1 to the softmax
  denominator, so the device subtracts a per-node pad count (exact correction).
  Zero-degree nodes get pad_count = D-1 so the denominator is exactly 1 and the
  output row is 0, matching segment_sum semantics.
- ALL compute stays on VectorE + ScalarE.  GPSIMD shares an SBUF port with
  VectorE; measured on HW, a DVE tensor_tensor slows down 7-9x while any GPSIMD
  tensor op runs, so offloading elementwise work to GPSIMD is a large net loss.
- Key columns are stored kk-major ([16 dk, 8 heads] per slot) so the dk-reduction
  of k*q is a 4-level pairwise-halving tree with contiguous 64/32/16/8-elem runs:
  every level runs in bf16 2x mode with a fresh compact destination (measured:
  in-place destinations cost ~+35%, tensor_reduce is 1.16-1.82 cyc/elem).
- Value columns are [cx(12), h(8)] per slot so the expw broadcast in the
  weighting multiply lands on a middle AP dim (2x mode).  The d-reduction does
  one fresh halving add, keeps halving in place while the running depth is even,
  and finishes with one strided reduce over the odd remainder.
- Normalization is one deferred wide pass at the end; output accumulates in SBUF
  and is stored with one DMA.
"""

import numpy as np

import concourse.bacc as bacc
import concourse.mybir as mybir
from concourse import tile
from concourse.bass_utils import run_bass_kernel_spmd

try:
    import ml_dtypes
    BF16_NP = np.dtype(ml_dtypes.bfloat16)
except ImportError:  # pragma: no cover
    BF16_NP = None

N_NODES = 50000
H = 8
P = 128  # nodes per block
N_CORES = 8
SCALE = float(1.0 / np.sqrt(128.0))
F32 = mybir.dt.float32

# key columns permuted from [h(8), kk(16)] to [kk(16), h(8)]
PERM_K = np.arange(128).reshape(8, 16).T.reshape(-1)
# value columns permuted from [h(8), cx(12)] to [cx(12), h(8)]
PERM_V = np.arange(96).reshape(8, 12).T.reshape(-1)
PERM_V_INV = np.argsort(PERM_V)


# ---------------------------------------------------------------- host prep

def prepare(value, key, query0, query1, edge_index, n_nodes=N_NODES, n_cores=N_CORES):
    """Build per-core padded ELL shards.  Returns (in_maps, meta)."""
    value = np.asarray(value, dtype=np.float32)
    key = np.asarray(key, dtype=np.float32)
    query0 = np.asarray(query0, dtype=np.float32)
    query1 = np.asarray(query1, dtype=np.float32)
    n_edges = key.shape[0]

    dst = np.asarray(edge_index[1], dtype=np.int64)
    deg = np.bincount(dst, minlength=n_nodes).astype(np.int64)
    n_pad = -(-n_nodes // (P * n_cores)) * (P * n_cores)  # round up to 1024
    deg_pad = np.concatenate([deg, np.zeros(n_pad - n_nodes, dtype=np.int64)])
    nb = n_pad // P
    ng = nb // n_cores

    order = np.argsort(deg_pad, kind="stable")  # node ids, degree-ascending
    degs_o = deg_pad[order]

    blk_max = degs_o.reshape(nb, P).max(axis=1)
    D_eff = np.maximum(blk_max.reshape(ng, n_cores).max(axis=1), 1).astype(np.int64)
    D_eff = (D_eff + 1) // 2 * 2  # even capacities for the halving add
    off = np.concatenate([[0], np.cumsum(P * D_eff)]).astype(np.int64)
    S = int(off[-1])  # slots per core

    pos = np.arange(n_pad)
    block = pos // P
    g_of = block // n_cores
    core_of = block % n_cores
    row = pos % P
    Dg = D_eff[g_of]
    base = off[g_of] + row * Dg

    edge_order = np.argsort(dst, kind="stable")
    starts = np.concatenate([[0], np.cumsum(deg)])

    pp = np.repeat(pos, degs_o)
    cum0 = np.concatenate([[0], np.cumsum(degs_o)])[:-1]
    d_idx = np.arange(n_edges) - np.repeat(cum0, degs_o)
    node_of_pp = order[pp]
    edge_ids = edge_order[starts[node_of_pp] + d_idx]
    slot_global = core_of[pp] * S + base[pp] + d_idx

    kp = np.zeros((n_cores * S, 128), dtype=np.float32)
    kp[slot_global] = key[:, PERM_K][edge_ids]
    vp = np.zeros((n_cores * S, 96), dtype=np.float32)
    vp[slot_global] = value.reshape(n_edges, 96)[:, PERM_V][edge_ids]
    kp = kp.reshape(n_cores, S, 128)
    vp = vp.reshape(n_cores, S, 96)

    qfull = np.concatenate([query0, query1], axis=-1).reshape(n_nodes, 128)[:, PERM_K]
    q_pad = np.zeros((n_pad, 128), dtype=np.float32)
    q_pad[:n_nodes] = qfull
    q_sorted = q_pad[order].reshape(nb, P, 128)

    pc = (Dg - degs_o).astype(np.float32)
    zero_deg = degs_o == 0
    pc[zero_deg] = (Dg[zero_deg] - 1).astype(np.float32)
    pc_sorted = pc.reshape(nb, P)

    dt = BF16_NP
    kp = kp.astype(dt)
    vp = vp.astype(dt)
    in_maps = []
    for c in range(n_cores):
        q_c = np.ascontiguousarray(
            q_sorted[c::n_cores].transpose(1, 0, 2).reshape(P, ng * 128)).astype(dt)
        pc_c = np.repeat(np.ascontiguousarray(pc_sorted[c::n_cores].T), H, axis=1)
        in_maps.append({"kp": kp[c], "vp": vp[c], "q": q_c, "pc": pc_c})

    meta = dict(D_eff=D_eff, off=off, S=S, NG=ng, NB=nb, order=order,
                n_nodes=n_nodes, n_pad=n_pad)
    return in_maps, meta


def unshard_output(out_cores, meta):
    """out_cores: list of [128, NG*96] -> [n_nodes, 32, 3]."""
    ng, nb = meta["NG"], meta["NB"]
    n_cores = len(out_cores)
    order, n_nodes, n_pad = meta["order"], meta["n_nodes"], meta["n_pad"]
    out_sorted = np.zeros((nb, P, 96), dtype=np.float32)
    for c in range(n_cores):
        out_sorted[c::n_cores] = (
            out_cores[c].reshape(P, ng, 96).transpose(1, 0, 2))
    out_sorted = out_sorted.reshape(n_pad, 96)[:, PERM_V_INV]
    out_full = np.zeros((n_nodes, 96), dtype=np.float32)
    mask = order < n_nodes
    out_full[order[mask]] = out_sorted[mask]
    return out_full.reshape(n_nodes, 32, 3)


# ---------------------------------------------------------------- bass kernel

def build(D_eff, S, NG, n_cores=N_CORES):
    D_eff = [int(d) for d in D_eff]
    off = np.concatenate([[0], np.cumsum([P * d for d in D_eff])]).astype(np.int64)

    nc = bacc.Bacc("TRN2", target_bir_lowering=False, debug=False,
                   num_devices=n_cores)
    DT = mybir.dt.bfloat16
    kp = nc.declare_dram_parameter("kp", [S, 128], DT, isOutput=False)
    vp = nc.declare_dram_parameter("vp", [S, 96], DT, isOutput=False)
    q = nc.declare_dram_parameter("q", [P, NG * 128], DT, isOutput=False)
    pc = nc.declare_dram_parameter("pc", [P, NG * H], F32, isOutput=False)
    out = nc.declare_dram_parameter("out", [P, NG * 96], F32, isOutput=True)

    mult = mybir.AluOpType.mult
    add = mybir.AluOpType.add
    AX = mybir.AxisListType.X

    with tile.TileContext(nc) as tc:
        with tc.tile_pool(name="res", bufs=1) as res, \
             tc.tile_pool(name="work", bufs=2) as work, \
             tc.tile_pool(name="small", bufs=3) as small:
            q_sb = res.tile([P, NG * 128], DT)
            nc.sync.dma_start(q_sb[:], q[:])
            pc_sb = res.tile([P, NG * H], F32)
            nc.sync.dma_start(pc_sb[:], pc[:])
            out_sb = res.tile([P, NG * 96], F32)
            ss_all = res.tile([P, NG * H], F32)

            for g in range(NG):
                D = D_eff[g]
                s0 = int(off[g])
                kt = work.tile([P, D * 128], DT, tag="kt")
                nc.sync.dma_start(
                    kt[:], kp[s0:s0 + P * D, :].rearrange("(n d) f -> n (d f)", n=P))
                vt = work.tile([P, D * 96], DT, tag="vt")
                nc.sync.dma_start(
                    vt[:], vp[s0:s0 + P * D, :].rearrange("(n d) f -> n (d f)", n=P))

                # w = k * q (q broadcast over d: outer stride-0, bf16 2x)
                qb = (q_sb[:, g * 128:(g + 1) * 128]
                      .unsqueeze(1).broadcast_to([P, D, 128]))
                w = work.tile([P, D * 128], DT, tag="kt")
                nc.vector.tensor_tensor(
                    out=w[:].rearrange("n (d f) -> n d f", d=D),
                    in0=kt[:].rearrange("n (d f) -> n d f", d=D),
                    in1=qb, op=mult)

                # dk-halving tree, kk-major so runs are 64/32/16/8 contiguous;
                # every level writes a fresh compact tile (bf16 2x)
                src, width = w, 128
                for X, tag in ((64, "t64"), (32, "t32"), (16, "t16")):
                    dstt = small.tile([P, D * X], DT, tag=tag)
                    sv = src[:].rearrange("n (d f) -> n d f", d=D)
                    nc.vector.tensor_tensor(
                        out=dstt[:].rearrange("n (d f) -> n d f", d=D),
                        in0=sv[:, :, :X], in1=sv[:, :, X:2 * X], op=add)
                    src, width = dstt, X
                lg = small.tile([P, D * H], DT, tag="lg")
                sv = src[:].rearrange("n (d f) -> n d f", d=D)
                nc.vector.tensor_tensor(
                    out=lg[:].rearrange("n (d h) -> n d h", d=D),
                    in0=sv[:, :, :8], in1=sv[:, :, 8:16], op=add)

                # expw = exp(scale * logits)  (ScalarE; contiguous)
                ew = small.tile([P, D * H], DT, tag="ew")
                nc.scalar.activation(out=ew[:], in_=lg[:],
                                     func=mybir.ActivationFunctionType.Exp,
                                     scale=SCALE)

                # segment sum of expw over d (strided reduce; small)
                nc.vector.tensor_reduce(
                    out=ss_all[:, g * H:(g + 1) * H],
                    in_=ew[:].rearrange("n (d h) -> n h d", d=D),
                    axis=AX, op=add)

                # wv = v * expw; v columns are [cx, h] so the expw broadcast is
                # on the middle dim and the inner stays contiguous (2x mode)
                wv = work.tile([P, D * 96], DT, tag="vt")
                eb = (ew[:].rearrange("n (d h) -> n d h", d=D)
                      .unsqueeze(2).broadcast_to([P, D, 12, H]))
                nc.vector.tensor_tensor(
                    out=wv[:].rearrange("n (d c h) -> n d c h", d=D, c=12),
                    in0=vt[:].rearrange("n (d c h) -> n d c h", d=D, c=12),
                    in1=eb, op=mult)

                # d-reduction: one fresh halving add, keep halving in place while
                # even, then one strided reduce over the odd remainder -> out_sb
                og = out_sb[:, g * 96:(g + 1) * 96]
                r = D // 2
                th = small.tile([P, r * 96], DT, tag="th")
                if r == 1:
                    nc.vector.tensor_tensor(
                        out=og, in0=wv[:, :96], in1=wv[:, 96:], op=add)
                else:
                    wv3 = wv[:].rearrange("n (d ch) -> n d ch", d=D)
                    nc.vector.tensor_tensor(
                        out=th[:].rearrange("n (d ch) -> n d ch", d=r),
                        in0=wv3[:, :r], in1=wv3[:, r:], op=add)
                    while r > 1 and r % 2 == 0:
                        r //= 2
                        if r == 1:
                            nc.vector.tensor_tensor(
                                out=og, in0=th[:, :96], in1=th[:, 96:192], op=add)
                        else:
                            nc.vector.tensor_tensor(
                                out=th[:, :r * 96], in0=th[:, :r * 96],
                                in1=th[:, r * 96:2 * r * 96], op=add)
                    if r > 1:
                        nc.vector.tensor_reduce(
                            out=og,
                            in_=th[:, :r * 96].rearrange("n (d c) -> n c d", d=r),
                            axis=AX, op=add)

            # one wide deferred normalization pass
            dn_all = res.tile([P, NG * H], F32)
            nc.vector.tensor_sub(out=dn_all[:], in0=ss_all[:], in1=pc_sb[:])
            rs_all = res.tile([P, NG * H], F32)
            nc.vector.reciprocal(out=rs_all[:], in_=dn_all[:])
            out2 = res.tile([P, NG * 96], F32)
            nc.vector.tensor_tensor(
                out=out2[:].rearrange("n (g c h) -> n g c h", g=NG, c=12),
                in0=out_sb[:].rearrange("n (g c h) -> n g c h", g=NG, c=12),
                in1=(rs_all[:].rearrange("n (g h) -> n g h", g=NG)
                     .unsqueeze(2).broadcast_to([P, NG, 12, H])),
                op=mult)

            nc.sync.dma_start(out[:], out2[:])

    nc.compile()
    return nc


# ---------------------------------------------------------------- entry point

LAST_RESULT = None  # BassKernelResults of the most recent run (for test harness)


def kernel(value, key, query0, query1, edge_index):
    global LAST_RESULT
    import os
    in_maps, meta = prepare(value, key, query0, query1, edge_index)
    nc = build(meta["D_eff"], meta["S"], meta["NG"])
    res = run_bass_kernel_spmd(nc, in_maps, list(range(N_CORES)),
                               tmpdir=os.environ.get("BASS_SPMD_TMPDIR"))
    LAST_RESULT = res
    out_cores = [res.results[c]["out"] for c in range(N_CORES)]
    return unshard_output(out_cores, meta)
